# revision 34
# baseline (speedup 1.0000x reference)
"""Trainium2 Bass kernel: windowed attention with dynamic positional bias.

out = softmax(scale*q@k^T + bias) @ v per (batch, head), where
bias[n,m,head] = pos_table[rel_idx[n,m], head] and rel_idx (faithfully to
the source) sums the three shifted relative coords: rel_idx = s(n) - s(m)
+ (h+w+d-3) with s(n) = sum of the 3D coords of position n.  Only rows
0..2*(h+w+d-3) of the MLP table are used and the MLP is row-wise, so the
tiny table (43x6 here) is computed on host and the bias is folded into
the QK matmul via augmented contraction channels:

  qaug[n] . kaug[m] = scale*q[n].k[m] + pos[s(n)-s(m)+OFF]     (exact)

with qaug = [qs_hi, qs_hi, qs_lo, pos[s(n)-b+OFF] over b] and
kaug = [k_hi, k_lo, k_hi, onehot(s(m))] — the hi/lo split keeps bf16
matmul inputs at ~fp32 logit accuracy (cross lo*lo term dropped).

Device per (b,h) pair: S^T tiles = kaug-block^T @ qaug (PE, K=128 bf16),
exp on ScalarE PSUM->SBUF bf16 (no max subtraction: |logits| < ~10), AV
with lhsT = exp(S^T) blocks and rhs = [v_hi, v_lo, 1] so the softmax
denominator falls out as an extra column, then VectorE reciprocal +
(hi+lo)*1/l.  Data parallel: 8 cores x 8 batches = 48 pairs/core.
"""

import sys

for _p in ("/opt/trn_rl_repo",):
    if _p not in sys.path:
        sys.path.insert(0, _p)

from contextlib import ExitStack

import ml_dtypes
import numpy as np

import concourse.bacc as bacc
import concourse.bass as bass
import concourse.tile as tile
from concourse import mybir
from concourse.bass_utils import run_bass_kernel_spmd

B, HEADS, HEAD_DIM = 64, 6, 32
NCORES = 8
BPC = B // NCORES              # batches per core
PAIRS = BPC * HEADS            # 48 (b,h) pairs per core
GROUP = 4                      # pairs per DMA group
NGROUPS = PAIRS // GROUP
N = 512                        # sequence positions (h*w*d)
VA = 2 * HEAD_DIM + 1          # [v_hi, v_lo, ones] columns

_BF16 = mybir.dt.bfloat16
_F32 = mybir.dt.float32


def _ln(x, g, b, eps=1e-5):
    mu = x.mean(axis=-1, keepdims=True)
    var = x.var(axis=-1, keepdims=True)
    return (x - mu) / np.sqrt(var + eps) * g + b


def _pos_table(h, w, d, pos_proj_w, pos_proj_b, ln1_g, ln1_b, w1, b1,
               ln2_g, ln2_b, w2, b2, ln3_g, ln3_b, w3, b3):
    bh = np.arange(1 - h, h, dtype=np.float32)
    bw = np.arange(1 - w, w, dtype=np.float32)
    bd = np.arange(1 - d, d, dtype=np.float32)
    mesh = np.stack(np.meshgrid(bh, bw, bd, indexing='ij')).reshape(3, -1).T
    x = mesh.astype(np.float32) @ pos_proj_w + pos_proj_b
    x = np.maximum(_ln(x, ln1_g, ln1_b), 0) @ w1 + b1
    x = np.maximum(_ln(x, ln2_g, ln2_b), 0) @ w2 + b2
    return (np.maximum(_ln(x, ln3_g, ln3_b), 0) @ w3 + b3).astype(np.float32)


def _build_device_program(loop_reps=None):
    """loop_reps: wrap the body in a device-side For_i (timing harness)."""
    nc = bacc.Bacc("TRN2", target_bir_lowering=False, debug=False)

    qf = PAIRS * N                 # qaug/kaug dram: [128, qf] bf16
    vf = PAIRS * 4 * VA            # vaug dram: [128, vf] bf16
    of = PAIRS * N                 # out dram: [VA, of] f32 (transposed, raw)

    qaug = nc.dram_tensor("qaug", [128, qf], _BF16, kind="ExternalInput").ap()
    kaug = nc.dram_tensor("kaug", [128, qf], _BF16, kind="ExternalInput").ap()
    vaug = nc.dram_tensor("vaug", [128, vf], _BF16, kind="ExternalInput").ap()
    out = nc.dram_tensor("out", [VA, of], _F32, kind="ExternalOutput").ap()

    qg_f = GROUP * N               # 2048
    vg_f = GROUP * 4 * VA          # 1040
    og_f = GROUP * N               # 2048

    with tile.TileContext(nc) as tc, ExitStack() as ctx:
        qpool = ctx.enter_context(tc.tile_pool(name="qg", bufs=3))
        kpool = ctx.enter_context(tc.tile_pool(name="kg", bufs=3))
        vpool = ctx.enter_context(tc.tile_pool(name="vg", bufs=3))
        ppool = ctx.enter_context(tc.tile_pool(name="pt", bufs=6))
        opool = ctx.enter_context(tc.tile_pool(name="og", bufs=2))
        spsum = ctx.enter_context(tc.tile_pool(name="spsum", bufs=3, space="PSUM"))
        apsum = ctx.enter_context(tc.tile_pool(name="apsum", bufs=2, space="PSUM"))

        # warmup exp so the ACT table load attaches to a dep-free
        # instruction (the first real exp otherwise exceeds the
        # per-instruction sync-wait slot limit in walrus codegen)
        wpool = ctx.enter_context(tc.tile_pool(name="warm", bufs=1))
        win = wpool.tile([128, 8], _F32, tag="win")
        wout = wpool.tile([128, 8], _F32, tag="wout")
        nc.vector.memset(win[:], 0.0)
        nc.scalar.activation(wout[:], win[:], mybir.ActivationFunctionType.Exp)

        import contextlib
        loop_cm = tc.For_i(0, loop_reps, 1) if loop_reps else contextlib.nullcontext()
        with loop_cm:
          # Software pipelining: emit pair p's AV matmuls AFTER pair p+1's
          # S^T matmuls so the in-order PE stream always has ready work
          # while ACT runs exp(p+1); otherwise AV(p) — which waits on
          # exp(p) — stalls the PE and delays exp(p+2)'s inputs.
          def emit_av(st):
              # AV, flipped: stationary = v chunk [128, 65] (tiny weight
              # load), moving = P^T chunk [128 (m), 512 (n)].  Accumulates
              # out^T = vaug.T @ P^T over the 4 m-chunks: rows 0-31 are
              # sum(P*v_hi), row 32 the softmax denominator (ones column),
              # rows 33-64 sum(P*v_lo).  Shipped raw; host does
              # (hi + lo) / l and the transpose back.
              vg, og, j, g, pts = st
              av = apsum.tile([VA, N], _F32)
              for m in range(4):
                  vcol = (4 * j + m) * VA
                  pth = pts[m // 2]
                  nc.tensor.matmul(
                      av[:, :],
                      lhsT=vg[:, vcol:vcol + VA],
                      rhs=pth[:, N * (m % 2):N * (m % 2) + N],
                      start=(m == 0), stop=(m == 3),
                  )
              nc.vector.tensor_copy(og[:, N * j:N * j + N], av[:, :])
              if j == GROUP - 1:
                  nc.sync.dma_start(out[:, g * og_f:(g + 1) * og_f], og[:])

          pending = None
          for g in range(NGROUPS):
              qg = qpool.tile([128, qg_f], _BF16)
              nc.sync.dma_start(qg[:], qaug[:, g * qg_f:(g + 1) * qg_f])
              kg = kpool.tile([128, qg_f], _BF16)
              nc.sync.dma_start(kg[:], kaug[:, g * qg_f:(g + 1) * qg_f])
              vg = vpool.tile([128, vg_f], _BF16)
              nc.sync.dma_start(vg[:], vaug[:, g * vg_f:(g + 1) * vg_f])
              og = opool.tile([VA, og_f], _F32)

              for j in range(GROUP):
                  fq = N * j
                  qa = qg[:, fq:fq + N]

                  # S^T in two halves (2 m-tiles each); exp each half to bf16
                  pts = []
                  for half in range(2):
                      sp = spsum.tile([128, 2 * N], _F32)
                      for mt in range(2):
                          m = 2 * half + mt
                          nc.tensor.matmul(
                              sp[:, N * mt:N * mt + N],
                              lhsT=kg[:, fq + 128 * m:fq + 128 * m + 128],
                              rhs=qa,
                              start=True, stop=True,
                          )
                      pt = ppool.tile([128, 2 * N], _BF16)
                      nc.scalar.activation(pt[:], sp[:],
                                           mybir.ActivationFunctionType.Exp)
                      pts.append(pt)

                  if pending is not None:
                      emit_av(pending)
                  pending = (vg, og, j, g, pts)

          emit_av(pending)

    nc.compile()
    return nc


def kernel(q, k, v, h, w, d,
           pos_proj_w, pos_proj_b,
           ln1_g, ln1_b, w1, b1,
           ln2_g, ln2_b, w2, b2,
           ln3_g, ln3_b, w3, b3):
    h, w, d = int(h), int(w), int(d)
    n = h * w * d
    assert n == N, f"kernel hardcoded for N={N}, got {n}"
    scale = np.float32(q.shape[-1] ** -0.5)

    q = np.asarray(q, np.float32)
    k = np.asarray(k, np.float32)
    v = np.asarray(v, np.float32)
    args = [np.asarray(a, np.float32) for a in (
        pos_proj_w, pos_proj_b, ln1_g, ln1_b, w1, b1,
        ln2_g, ln2_b, w2, b2, ln3_g, ln3_b, w3, b3)]
    pos = _pos_table(h, w, d, *args)

    coords = np.stack(np.meshgrid(np.arange(h), np.arange(w), np.arange(d),
                                  indexing='ij')).reshape(3, -1)
    s = coords.sum(axis=0)
    s_max = (h - 1) + (w - 1) + (d - 1)
    naug = s_max + 1                           # 22
    assert 3 * HEAD_DIM + naug <= 128
    bidx = np.arange(naug)

    bf = ml_dtypes.bfloat16
    Qrows = pos[(s[:, None] - bidx[None, :]) + s_max, :]     # (N, naug, HEADS)
    E = (s[:, None] == bidx[None, :]).astype(np.float32)     # (N, naug)

    qs = q * scale
    q_hi = qs.astype(bf)
    q_lo = (qs - q_hi.astype(np.float32)).astype(bf)
    k_hi = k.astype(bf)
    k_lo = (k - k_hi.astype(np.float32)).astype(bf)
    v_hi = v.astype(bf)
    v_lo = (v - v_hi.astype(np.float32)).astype(bf)

    D = HEAD_DIM
    qaug_all = np.zeros((B, HEADS, 128, N), dtype=bf)
    qaug_all[:, :, 0:D] = q_hi.transpose(0, 1, 3, 2)
    qaug_all[:, :, D:2 * D] = q_hi.transpose(0, 1, 3, 2)
    qaug_all[:, :, 2 * D:3 * D] = q_lo.transpose(0, 1, 3, 2)
    qaug_all[:, :, 3 * D:3 * D + naug] = Qrows.transpose(2, 1, 0).astype(bf)[None]
    kaug_all = np.zeros((B, HEADS, 128, N), dtype=bf)
    kaug_all[:, :, 0:D] = k_hi.transpose(0, 1, 3, 2)
    kaug_all[:, :, D:2 * D] = k_lo.transpose(0, 1, 3, 2)
    kaug_all[:, :, 2 * D:3 * D] = k_hi.transpose(0, 1, 3, 2)
    kaug_all[:, :, 3 * D:3 * D + naug] = E.T.astype(bf)[None, None]
    vaug_all = np.ones((B, HEADS, N, VA), dtype=bf)
    vaug_all[:, :, :, 0:D] = v_hi
    vaug_all[:, :, :, D + 1:] = v_lo          # col D is the ones column

    def pack_qk(a):   # [PAIRS, 128, N] -> [128, PAIRS*N], pair-major free
        return np.ascontiguousarray(a.transpose(1, 0, 2).reshape(128, -1))

    def pack_v(a):    # [PAIRS, N, VA] -> [128, PAIRS*4*VA], chunk-major free
        return np.ascontiguousarray(
            a.reshape(PAIRS * 4, 128, VA).transpose(1, 0, 2).reshape(128, -1))

    in_maps = []
    for c in range(NCORES):
        sl = slice(c * BPC, (c + 1) * BPC)
        in_maps.append({
            "qaug": pack_qk(qaug_all[sl].reshape(PAIRS, 128, N)),
            "kaug": pack_qk(kaug_all[sl].reshape(PAIRS, 128, N)),
            "vaug": pack_v(vaug_all[sl].reshape(PAIRS, N, VA)),
        })

    nc = _build_device_program()
    res = run_bass_kernel_spmd(nc, in_maps, list(range(NCORES)))

    out = np.empty((B, HEADS, n, HEAD_DIM), np.float32)
    for c in range(NCORES):
        oc = np.asarray(res.results[c]["out"])           # [VA, PAIRS*N] raw
        arr = oc.reshape(VA, PAIRS, n).transpose(1, 0, 2)  # [PAIRS, VA, n]
        norm = (arr[:, :HEAD_DIM] + arr[:, HEAD_DIM + 1:]) / arr[:, HEAD_DIM:HEAD_DIM + 1]
        out[c * BPC:(c + 1) * BPC] = (
            norm.transpose(0, 2, 1).reshape(BPC, HEADS, n, HEAD_DIM))
    return out

